# revision 22
# baseline (speedup 1.0000x reference)
"""Trainium2 Bass kernel for nn_BICEPNeuralLayer.

Math: the reference module (Euler-Maruyama SDE scan -> Conv1d over time ->
time-mean -> linear projection) is LINEAR in the noise tensor, so the whole
pipeline collapses algebraically:

  paths[t] = c_b * sum_s retain^(t-s) eps_s          (c_b = feedback_b*sqrt(dt))
  mean_t(conv(paths)) folds to per-timestep weights on eps:
     out[b] = (c_b/NS) * (Tsum @ A[b] - T0 @ L[b] - T2 @ F[b]) + bias
  A[b,i] = sum_s gA[s] noise[b,s,i],   gA[s] = (1-retain^(NS-s))/(1-retain)
  L[b,i] = sum_s retain^(NS-1-s) noise[b,s,i]
  F[b,i] = noise[b,0,i]
  Tsum = out_w @ (W0+W1+W2), T0 = out_w @ W0, T2 = out_w @ W2  (Wk = conv_w[:,:,k])
  bias  = out_w @ conv_b + out_b

Device work per core (pure data parallel over batch, 32 samples/core).
The kernel is HBM-stream-bound (~11.5 MB/core at ~400 GB/s effective), so
everything is organized around one continuously-busy DMA queue:

  - noise is host-pre-transposed to chunk-major [q][s][b][i] so every DMA
    descriptor is a long sequential DRAM run; feature chunks 0-5 are packed
    in PAIRS (16 KB/partition/descriptor) to cut per-descriptor overhead,
    chunks 6 (128 wide) and 7 (104 wide) stay single so the tail-gating
    transfers are small. No feature padding (P=1000 = 3*256 + 128 + 104).
  - the three folded [OUT, P] matrices (mcat) are coalesced into two large
    slices early and two small per-chunk slices at the end, interleaved so
    stage-2 for chunk q never waits and the final slice only gates the last
    3 accumulating matmuls.
  - all small constants ride one fp16 [128, 611] block (g3 | cvec | bias),
    a single line-rate transfer (tiny broadcast transfers otherwise stall
    the ring at <40 GB/s).
  - bias enters the output psum as an early K=1 matmul (ones.T @ bias_row)
    so the final psum->sbuf stores are pure casts split across DVE and ACT.
  - a 10-matmul warmup burst on zeroed scratch runs during the initial DMA
    window (PE HAM clock-gate reaches 2.4 GHz only after ~3.4 us busy).

  per chunk q (software-pipelined: s1(0), s1(1), s2(0), s1(2), s2(1), ...):
    stage 1: 32 matmuls lhsT=noise[q][:,b,:] (fp16, FWL) rhs=g3[128,3]
             -> psum[i, (b,{A,L,F})]
    V build: DVE reorder (b,v)->(v,b) fused with the per-sample feedback
             scale c_b (host-precomputed sigmoid)
    stage 2: 3 accumulating matmuls lhsT=V[128i x 32b] rhs=mcat[128i x 512j]
             -> psum[32b, 512j]
"""

import sys

if "/opt/trn_rl_repo" not in sys.path:
    sys.path.insert(0, "/opt/trn_rl_repo")

from contextlib import ExitStack

import numpy as np

import concourse.bass as bass
import concourse.tile as tile
from concourse import mybir
from concourse.bass_utils import run_bass_kernel_spmd

B, IN, OUT, P, NS = 256, 1024, 512, 1000, 128
NCORES = 8
BSH = B // NCORES   # 32 samples per core
NQ = 8              # feature chunks: 3 pairs of 256 + 128 + 104 = 1000
W7 = P - 7 * 128    # 104
NWARM = 10          # HAM warmup matmuls (N=512 each)

F32 = mybir.dt.float32
F16 = mybir.dt.float16
F16_NP = mybir.dt.np(F16)

_CACHE = {}

LAST_RUN = None  # BassKernelResults of the most recent execution (for test.py)


def _chunk_w(q):
    return 128 if q < 7 else W7


def _split_sync_waits(nc: bass.Bass, max_waits: int = 1) -> int:
    """Walrus in this container accepts at most one sync-wait command per
    instruction. Tile emits instructions (notably the epilogue Drain and any
    op depending on two DMA queues) with several waits. Split the surplus
    onto single-wait NoOps inserted just before, on the same engine, which
    is semantically identical for sem-ge waits."""
    nid = 0
    for fn in nc.m.functions:
        for bb in fn.blocks:
            insts = list(bb.instructions)
            out, changed = [], False
            for inst in insts:
                si = inst.sync_info
                if si is not None and si.on_wait and len(si.on_wait) > max_waits:
                    waits = list(si.on_wait)
                    extra, keep = waits[:-max_waits], waits[-max_waits:]
                    for w in extra:
                        nid += 1
                        out.append(
                            mybir.InstNoOp(
                                name=f"waitsplit-{nid}",
                                sync_info=mybir.SyncInfo(on_wait=[w], on_update=[]),
                                bass_nofuse=True,
                                engine=inst.engine,
                            )
                        )
                    inst.sync_info = mybir.SyncInfo(
                        on_wait=keep, on_update=list(si.on_update)
                    )
                    changed = True
                out.append(inst)
            if changed:
                bb.instructions = out
    return nid


def _build_program() -> bass.Bass:
    if "nc" in _CACHE:
        return _CACHE["nc"]

    nc = bass.Bass()

    noise_p = nc.dram_tensor("noise_p", [3, NS, BSH, 256], F16,
                             kind="ExternalInput")
    noise_s6 = nc.dram_tensor("noise_s6", [NS, BSH, 128], F16,
                              kind="ExternalInput")
    noise_s7 = nc.dram_tensor("noise_s7", [NS, BSH, W7], F16,
                              kind="ExternalInput")
    # packed consts block (fp16): cols [0:3] g3, [3:99] cvec broadcast,
    # [99:611] bias (meaningful on partition 0)
    cblk_d = nc.dram_tensor("cblk", [128, 611], F16, kind="ExternalInput")
    mcat_d = nc.dram_tensor("mcat", [128, 3 * NQ, OUT], F16, kind="ExternalInput")
    out_d = nc.dram_tensor("out", [BSH, OUT], F16, kind="ExternalOutput")

    with ExitStack() as ctx:
        tc = ctx.enter_context(tile.TileContext(nc))
        consts = ctx.enter_context(tc.tile_pool(name="consts", bufs=1))
        np_pairs = ctx.enter_context(tc.tile_pool(name="npair", bufs=3))
        np_sing = ctx.enter_context(tc.tile_pool(name="nsing", bufs=2))
        vpool = ctx.enter_context(tc.tile_pool(name="v", bufs=1))
        ps1 = ctx.enter_context(tc.tile_pool(name="ps1", bufs=4, space="PSUM"))
        ps2 = ctx.enter_context(tc.tile_pool(name="ps2", bufs=1, space="PSUM"))
        wps = ctx.enter_context(tc.tile_pool(name="wps", bufs=1, space="PSUM"))

        # ---- HAM warmup: zeroed scratch matmuls, no data dependencies ----
        warm_sb = consts.tile([128, 512], F16, tag="warm")
        nc.vector.memset(warm_sb[:], 0.0)
        warm_ps = wps.tile([128, 512], F32, tag="warmps")
        for _ in range(NWARM):
            nc.tensor.matmul(warm_ps[:], lhsT=warm_sb[:, 0:128], rhs=warm_sb[:],
                             start=True, stop=True)

        # ---- consts: one line-rate transfer, first on the SP ring ----
        cblk_sb = consts.tile([128, 611], F16, tag="cblk")
        nc.sync.dma_start(out=cblk_sb[:], in_=cblk_d[:])
        g3_sb = cblk_sb[:, 0:3]
        c_sb = cblk_sb[:, 3:99]
        bias16 = cblk_sb[0:1, 99:611]
        ones_sb = consts.tile([1, BSH], F16, tag="ones")
        nc.vector.memset(ones_sb[:], 1.0)

        # ---- the big stream on the SP ring: noise chunk-pairs (2.1 MB,
        # 16 KB/partition descriptors) with coalesced mcat slices between,
        # then the two small tail chunks with per-chunk mcat slices ----
        mcat_sb = consts.tile([128, 3 * NQ, OUT], F16, tag="mcat")
        pair_t = [np_pairs.tile([NS, BSH, 256], F16, name=f"npair{p}",
                                tag="npair") for p in range(3)]
        s6_t = np_sing.tile([NS, BSH, 128], F16, name="n6", tag="nsing")
        s7_t = np_sing.tile([NS, BSH, W7], F16, name="n7", tag="nsing")

        nc.sync.dma_start(out=pair_t[0][:], in_=noise_p[0])
        nc.sync.dma_start(out=mcat_sb[:, 0:9, :], in_=mcat_d[:][:, 0:9, :])
        nc.sync.dma_start(out=pair_t[1][:], in_=noise_p[1])
        nc.sync.dma_start(out=mcat_sb[:, 9:18, :], in_=mcat_d[:][:, 9:18, :])
        nc.sync.dma_start(out=pair_t[2][:], in_=noise_p[2])
        nc.sync.dma_start(out=s6_t[:], in_=noise_s6[:])
        nc.sync.dma_start(out=mcat_sb[:, 18:21, :], in_=mcat_d[:][:, 18:21, :])
        nc.sync.dma_start(out=s7_t[:], in_=noise_s7[:])
        nc.sync.dma_start(out=mcat_sb[:, 21:24, :], in_=mcat_d[:][:, 21:24, :])

        def noise_ap(q, b):
            if q < 6:
                off = (q % 2) * 128
                return pair_t[q // 2][:, b, off : off + 128]
            if q == 6:
                return s6_t[:, b, :]
            return s7_t[:, b, :]

        # ---- per-chunk pipeline, software-pipelined by one chunk so the
        # DVE V-build of chunk q overlaps stage-1 of chunk q+1 ----
        ps_out = ps2.tile([BSH, OUT], F32, tag="ps2")
        v_t = [vpool.tile([128, 3 * BSH], F16, name=f"v{q}", tag=f"v{q}")
               for q in range(NQ)]

        def stage1(q):
            w = _chunk_w(q)
            pt = ps1.tile([128, 3 * BSH], F32, name=f"ps1_{q}", tag="ps1")
            for b in range(BSH):
                nc.tensor.matmul(
                    pt[0:w, b * 3 : b * 3 + 3],
                    lhsT=noise_ap(q, b),
                    rhs=g3_sb,
                    start=True,
                    stop=True,
                )
            # psum -> V (fp16): reorder (b,v) -> (v,b) and fold the
            # per-sample feedback scale c_b in (c columns follow V layout)
            src = pt[0:w, :].rearrange("p (b v) -> p v b", v=3)
            dst = v_t[q][0:w, :].rearrange("p (v b) -> p v b", v=3)
            csrc = c_sb[0:w, :].rearrange("p (v b) -> p v b", v=3)
            nc.vector.tensor_mul(dst, src, csrc)

        def stage2(q):
            w = _chunk_w(q)
            for v in range(3):
                t = q * 3 + v
                nc.tensor.matmul(
                    ps_out[:],
                    lhsT=v_t[q][0:w, v * BSH : (v + 1) * BSH],
                    rhs=mcat_sb[0:w, t, :],
                    start=False,
                    stop=(t == 3 * NQ - 1),
                )

        # bias into ps_out (opens the accumulation group)
        nc.tensor.matmul(ps_out[:], lhsT=ones_sb[:], rhs=bias16,
                         start=True, stop=False)

        stage1(0)
        for q in range(1, 6):
            stage1(q)
            stage2(q - 1)
        stage2(5)
        stage1(6)
        stage2(6)
        stage1(7)
        stage2(7)

        # ---- store fp16 (bias already accumulated in psum): pure casts,
        # split DVE/ACT so the two halves overlap ----
        out_sb = consts.tile([BSH, OUT], F16, tag="outsb")
        nc.vector.tensor_scalar_add(out_sb[:, 0:256], ps_out[:, 0:256], 0.0)
        nc.scalar.copy(out_sb[:, 256:512], ps_out[:, 256:512])
        nc.sync.dma_start(out=out_d[:], in_=out_sb[:])

    _split_sync_waits(nc)
    _CACHE["nc"] = nc
    return nc


def _host_precompute(decay_param, conv_w, conv_b, out_w, out_b):
    dp = float(np.asarray(decay_param).reshape(-1)[0])
    decay = 0.5 / (1.0 + np.exp(-dp))
    dt = 1.0 / NS
    retain = 1.0 - decay * dt

    s = np.arange(NS, dtype=np.float64)
    gA = (1.0 - retain ** (NS - s)) / (1.0 - retain)
    gL = retain ** (NS - 1 - s)
    g3 = np.zeros((NS, 3), np.float32)
    g3[:, 0] = gA
    g3[:, 1] = gL
    g3[0, 2] = 1.0

    conv_w = np.asarray(conv_w, np.float32)
    out_w = np.asarray(out_w, np.float32)
    w_sum = conv_w.sum(axis=2)
    t_sum = out_w @ w_sum              # [OUT, P]
    t0 = out_w @ conv_w[:, :, 0]
    t2 = out_w @ conv_w[:, :, 2]
    r = np.stack([t_sum, -t0, -t2])    # [3, OUT, P]
    r_pad = np.zeros((3, OUT, NQ * 128), np.float32)
    r_pad[:, :, :P] = r
    # mcat[p, q*3+v, j] = r[v, j, q*128+p]  (q-major: per-chunk slices)
    mcat = r_pad.reshape(3, OUT, NQ, 128).transpose(3, 2, 0, 1)  # [128, NQ, 3, OUT]
    mcat = np.ascontiguousarray(mcat.reshape(128, 3 * NQ, OUT).astype(F16_NP))

    bias_vec = (
        out_w @ np.asarray(conv_b, np.float32)
        + np.asarray(out_b, np.float32).reshape(OUT)
    )
    return g3, mcat, bias_vec


def kernel(x, noise, fb_w, fb_b, decay_param, conv_w, conv_b, out_w, out_b,
           _trace=False):
    global LAST_RUN

    x = np.asarray(x, np.float32)
    # chunk-major noise layouts, sequential DRAM runs per partition:
    #   pairs: chunks 0-5 as [core][pair][s][b][256] (16 KB/partition runs)
    #   singles: chunk 6 [core][s][b][128], chunk 7 [core][s][b][104]
    n16 = np.asarray(noise, np.float32).astype(F16_NP)
    n16 = n16.reshape(NCORES, BSH, NS, P)
    noise_pair = np.ascontiguousarray(
        n16[:, :, :, : 3 * 256].reshape(NCORES, BSH, NS, 3, 256)
        .transpose(0, 3, 2, 1, 4)
    )  # [NCORES, 3, NS, BSH, 256]
    noise_s6 = np.ascontiguousarray(
        n16[:, :, :, 768:896].transpose(0, 2, 1, 3))  # [NCORES, NS, BSH, 128]
    noise_s7 = np.ascontiguousarray(
        n16[:, :, :, 896:P].transpose(0, 2, 1, 3))    # [NCORES, NS, BSH, 104]

    g3, mcat, bias_vec = _host_precompute(decay_param, conv_w, conv_b, out_w, out_b)

    # per-sample feedback scale: sigmoid(x . fb_w + fb_b) * sqrt(dt)/NS
    fb_w = np.asarray(fb_w, np.float32).reshape(IN)
    fb_b = float(np.asarray(fb_b, np.float32).reshape(-1)[0])
    z = x @ fb_w + fb_b
    cvec = (1.0 / (1.0 + np.exp(-z, dtype=np.float64))) * (np.sqrt(1.0 / NS) / NS)
    cvec = cvec.reshape(B).astype(np.float32)

    nc = _build_program()

    in_maps = []
    for c in range(NCORES):
        sl = slice(c * BSH, (c + 1) * BSH)
        cblk = np.zeros((128, 611), F16_NP)
        cblk[:, 0:3] = g3
        cblk[:, 3:99] = np.tile(cvec[sl], 3).reshape(1, 96)
        cblk[0, 99:611] = bias_vec
        in_maps.append(
            {
                "noise_p": noise_pair[c],
                "noise_s6": noise_s6[c],
                "noise_s7": noise_s7[c],
                "cblk": np.ascontiguousarray(cblk),
                "mcat": mcat,
            }
        )

    res = run_bass_kernel_spmd(nc, in_maps, core_ids=list(range(NCORES)),
                               trace=_trace)
    LAST_RUN = res
    out = np.concatenate([m["out"] for m in res.results], axis=0)
    return out.astype(np.float32)


# revision 25
# speedup vs baseline: 1.0063x; 1.0063x over previous
"""Trainium2 Bass kernel for nn_BICEPNeuralLayer.

Math: the reference module (Euler-Maruyama SDE scan -> Conv1d over time ->
time-mean -> linear projection) is LINEAR in the noise tensor, so the whole
pipeline collapses algebraically:

  paths[t] = c_b * sum_s retain^(t-s) eps_s          (c_b = feedback_b*sqrt(dt))
  mean_t(conv(paths)) folds to per-timestep weights on eps:
     out[b] = (c_b/NS) * (Tsum @ A[b] - T0 @ L[b] - T2 @ F[b]) + bias
  A[b,i] = sum_s gA[s] noise[b,s,i],   gA[s] = (1-retain^(NS-s))/(1-retain)
  L[b,i] = sum_s retain^(NS-1-s) noise[b,s,i]
  F[b,i] = noise[b,0,i]
  Tsum = out_w @ (W0+W1+W2), T0 = out_w @ W0, T2 = out_w @ W2  (Wk = conv_w[:,:,k])
  bias  = out_w @ conv_b + out_b

Device work per core (pure data parallel over batch, 32 samples/core):
  The noise shard is pre-transposed on the host to chunk-major layout
  [q][s][b][i] (i padded 1000->1024, 8 chunks of 128 features) so every DMA
  descriptor is an 8 KB sequential DRAM run (~410 GB/s vs ~295 GB/s for the
  strided [b][s][i] order). mcat (the three folded [OUT, P] matrices) is cut
  into per-chunk slices interleaved with the noise chunks on the same queue,
  so stage 2 accumulates chunk-by-chunk behind the stream instead of
  serializing at the end. All small constants are host-pre-broadcast into a
  single [128, 611] fp32 block = one line-rate DMA (tiny broadcast transfers
  otherwise stall the ring for ~2 us at <40 GB/s).

  A 10-matmul warmup burst on zeroed scratch runs during the initial DMA
  window: the PE HAM clock-gate only reaches 2.4 GHz after ~3.4 us of
  sustained busy, and the real per-chunk bursts are too short to ever get
  there on their own (everything measured 2x slow at 1.2 GHz without this).

  per chunk q (software-pipelined: s1(0), s1(1), s2(0), s1(2), s2(1), ...):
    stage 1: 32 matmuls lhsT=noise[q][:,b,:] (fp16, FWL) rhs=g3[128,3]
             -> psum[i, (b,{A,L,F})]
    V build: DVE reorder (b,v)->(v,b) fused with the per-sample feedback
             scale c_b (host-precomputed sigmoid)
    stage 2: 3 accumulating matmuls lhsT=V[128i x 32b] rhs=mcat[128i x 512j]
             -> psum[32b, 512j]
  tail: add bias, store [32, 512] fp16 (host upcasts).
"""

import sys

if "/opt/trn_rl_repo" not in sys.path:
    sys.path.insert(0, "/opt/trn_rl_repo")

from contextlib import ExitStack

import numpy as np

import concourse.bass as bass
import concourse.tile as tile
from concourse import mybir
from concourse.bass_utils import run_bass_kernel_spmd

B, IN, OUT, P, NS = 256, 1024, 512, 1000, 128
NCORES = 8
BSH = B // NCORES  # 32 samples per core
NQ = 8             # feature chunks: 7 x 128 + 104 = 1000 (no padding)
W7 = P - 7 * 128   # 104
NWARM = 10         # HAM warmup matmuls (N=512 each)


def _chunk_w(q):
    return 128 if q < 7 else W7

F32 = mybir.dt.float32
F16 = mybir.dt.float16
F16_NP = mybir.dt.np(F16)

_CACHE = {}

LAST_RUN = None  # BassKernelResults of the most recent execution (for test.py)


def _split_sync_waits(nc: bass.Bass, max_waits: int = 1) -> int:
    """Walrus in this container accepts at most one sync-wait command per
    instruction. Tile emits instructions (notably the epilogue Drain and any
    op depending on two DMA queues) with several waits. Split the surplus
    onto single-wait NoOps inserted just before, on the same engine, which
    is semantically identical for sem-ge waits."""
    nid = 0
    for fn in nc.m.functions:
        for bb in fn.blocks:
            insts = list(bb.instructions)
            out, changed = [], False
            for inst in insts:
                si = inst.sync_info
                if si is not None and si.on_wait and len(si.on_wait) > max_waits:
                    waits = list(si.on_wait)
                    extra, keep = waits[:-max_waits], waits[-max_waits:]
                    for w in extra:
                        nid += 1
                        out.append(
                            mybir.InstNoOp(
                                name=f"waitsplit-{nid}",
                                sync_info=mybir.SyncInfo(on_wait=[w], on_update=[]),
                                bass_nofuse=True,
                                engine=inst.engine,
                            )
                        )
                    inst.sync_info = mybir.SyncInfo(
                        on_wait=keep, on_update=list(si.on_update)
                    )
                    changed = True
                out.append(inst)
            if changed:
                bb.instructions = out
    return nid


def _build_program() -> bass.Bass:
    if "nc" in _CACHE:
        return _CACHE["nc"]

    nc = bass.Bass()

    noise_d = nc.dram_tensor("noise_sh", [NQ - 1, NS, BSH, 128], F16,
                             kind="ExternalInput")
    noise_s7 = nc.dram_tensor("noise_s7", [NS, BSH, W7], F16,
                              kind="ExternalInput")
    # packed consts block (fp16): cols [0:3] g3, [3:99] cvec broadcast,
    # [99:611] bias (meaningful on partition 0)
    cblk_d = nc.dram_tensor("cblk", [128, 611], F16, kind="ExternalInput")
    mcat_d = nc.dram_tensor("mcat", [128, 3 * NQ, OUT], F16, kind="ExternalInput")
    out_d = nc.dram_tensor("out", [BSH, OUT], F16, kind="ExternalOutput")

    with ExitStack() as ctx:
        tc = ctx.enter_context(tile.TileContext(nc))
        consts = ctx.enter_context(tc.tile_pool(name="consts", bufs=1))
        npool = ctx.enter_context(tc.tile_pool(name="noise", bufs=NQ))
        vpool = ctx.enter_context(tc.tile_pool(name="v", bufs=1))
        ps1 = ctx.enter_context(tc.tile_pool(name="ps1", bufs=4, space="PSUM"))
        ps2 = ctx.enter_context(tc.tile_pool(name="ps2", bufs=1, space="PSUM"))
        wps = ctx.enter_context(tc.tile_pool(name="wps", bufs=1, space="PSUM"))

        # ---- HAM warmup: zeroed scratch matmuls, no data dependencies ----
        warm_sb = consts.tile([128, 512], F16, tag="warm")
        nc.vector.memset(warm_sb[:], 0.0)
        warm_ps = wps.tile([128, 512], F32, tag="warmps")
        for _ in range(NWARM):
            nc.tensor.matmul(warm_ps[:], lhsT=warm_sb[:, 0:128], rhs=warm_sb[:],
                             start=True, stop=True)

        # ---- consts: one line-rate fp16 transfer, first on the SP ring.
        # bias enters ps_out as an early K=1 matmul (ones.T @ bias_row), so
        # the final psum->sbuf stores are pure casts split across DVE/ACT
        cblk_sb = consts.tile([128, 611], F16, tag="cblk")
        nc.sync.dma_start(out=cblk_sb[:], in_=cblk_d[:])
        g3_sb = cblk_sb[:, 0:3]
        c_sb = cblk_sb[:, 3:99]
        bias16 = cblk_sb[0:1, 99:611]
        ones_sb = consts.tile([1, BSH], F16, tag="ones")
        nc.vector.memset(ones_sb[:], 1.0)

        # ---- the big stream: noise chunk q (1 MB, fully sequential in DRAM)
        # interleaved with mcat slice q (393 KB) on the SP ring. Stage-2 for
        # chunk q only needs bytes that arrived with chunk q, so compute
        # chases the stream and almost nothing is left after the last byte.
        mcat_sb = consts.tile([128, 3 * NQ, OUT], F16, tag="mcat")
        noise_t = []
        HB = BSH // 2
        for q in range(NQ):
            w = _chunk_w(q)
            t = npool.tile([NS, BSH, w], F16, name=f"noise{q}", tag="noise")
            # mcat slice q AFTER noise chunk q: the last chunk's V-build path
            # then starts as early as possible and the final mcat slice's
            # completion receipt hides under it.
            nc.sync.dma_start(out=t[:], in_=noise_d[q] if q < 7 else noise_s7[:])
            noise_t.append(t)
            nc.sync.dma_start(out=mcat_sb[:, q * 3 : (q + 1) * 3, :],
                              in_=mcat_d[:][:, q * 3 : (q + 1) * 3, :])

        # ---- per-chunk pipeline, software-pipelined by one chunk: the PE
        # stream is s1(0), s1(1), s2(0), s1(2), s2(1), ... so the DVE V-build
        # of chunk q overlaps stage-1 of chunk q+1 instead of stalling the PE.
        ps_out = ps2.tile([BSH, OUT], F32, tag="ps2")
        v_t = [vpool.tile([128, 3 * BSH], F16, name=f"v{q}", tag=f"v{q}")
               for q in range(NQ)]

        def stage1(q):
            w = _chunk_w(q)
            pt = ps1.tile([128, 3 * BSH], F32, name=f"ps1_{q}", tag="ps1")
            for b in range(BSH):
                nc.tensor.matmul(
                    pt[0:w, b * 3 : b * 3 + 3],
                    lhsT=noise_t[q][:, b, :],
                    rhs=g3_sb,
                    start=True,
                    stop=True,
                )
            # psum -> V (fp16): reorder (b,v) -> (v,b) and fold the
            # per-sample feedback scale c_b in (c columns follow V layout)
            src = pt[0:w, :].rearrange("p (b v) -> p v b", v=3)
            dst = v_t[q][0:w, :].rearrange("p (v b) -> p v b", v=3)
            csrc = c_sb[0:w, :].rearrange("p (v b) -> p v b", v=3)
            nc.vector.tensor_mul(dst, src, csrc)

        def stage2(q):
            w = _chunk_w(q)
            for v in range(3):
                t = q * 3 + v
                nc.tensor.matmul(
                    ps_out[:],
                    lhsT=v_t[q][0:w, v * BSH : (v + 1) * BSH],
                    rhs=mcat_sb[0:w, t, :],
                    start=False,
                    stop=(t == 3 * NQ - 1),
                )

        # bias into ps_out (opens the accumulation group)
        nc.tensor.matmul(ps_out[:], lhsT=ones_sb[:], rhs=bias16[:],
                         start=True, stop=False)

        stage1(0)
        for q in range(1, NQ):
            stage1(q)
            stage2(q - 1)
        stage2(NQ - 1)

        # ---- store fp16 (bias already accumulated in psum): pure casts,
        # split DVE/ACT so the two halves overlap ----
        out_sb = consts.tile([BSH, OUT], F16, tag="outsb")
        nc.vector.tensor_scalar_add(out_sb[:, 0:256], ps_out[:, 0:256], 0.0)
        nc.scalar.copy(out_sb[:, 256:512], ps_out[:, 256:512])
        nc.sync.dma_start(out=out_d[:], in_=out_sb[:])

    _split_sync_waits(nc)
    _CACHE["nc"] = nc
    return nc


def _host_precompute(decay_param, conv_w, conv_b, out_w, out_b):
    dp = float(np.asarray(decay_param).reshape(-1)[0])
    decay = 0.5 / (1.0 + np.exp(-dp))
    dt = 1.0 / NS
    retain = 1.0 - decay * dt

    s = np.arange(NS, dtype=np.float64)
    gA = (1.0 - retain ** (NS - s)) / (1.0 - retain)
    gL = retain ** (NS - 1 - s)
    g3 = np.zeros((NS, 3), np.float32)
    g3[:, 0] = gA
    g3[:, 1] = gL
    g3[0, 2] = 1.0

    conv_w = np.asarray(conv_w, np.float32)
    out_w = np.asarray(out_w, np.float32)
    w_sum = conv_w.sum(axis=2)
    t_sum = out_w @ w_sum              # [OUT, P]
    t0 = out_w @ conv_w[:, :, 0]
    t2 = out_w @ conv_w[:, :, 2]
    r = np.stack([t_sum, -t0, -t2])    # [3, OUT, P]
    r_pad = np.zeros((3, OUT, NQ * 128), np.float32)
    r_pad[:, :, :P] = r
    # mcat[p, q*3+v, j] = r[v, j, q*128+p]  (q-major: per-chunk slices)
    mcat = r_pad.reshape(3, OUT, NQ, 128).transpose(3, 2, 0, 1)  # [128, NQ, 3, OUT]
    mcat = np.ascontiguousarray(mcat.reshape(128, 3 * NQ, OUT).astype(F16_NP))

    bias_vec = (
        out_w @ np.asarray(conv_b, np.float32)
        + np.asarray(out_b, np.float32).reshape(OUT)
    )
    return g3, mcat, bias_vec


def kernel(x, noise, fb_w, fb_b, decay_param, conv_w, conv_b, out_w, out_b,
           _trace=False):
    global LAST_RUN

    x = np.asarray(x, np.float32)
    # chunk-major, per-core noise layout [core][q][s][b][i]: every DMA
    # descriptor reads a multi-KB sequential DRAM run. Chunk 7 is 104 wide
    # (P = 7*128 + 104, no padding).
    n16 = np.asarray(noise, np.float32).astype(F16_NP).reshape(NCORES, BSH, NS, P)
    noise_q = np.ascontiguousarray(
        n16[:, :, :, : 7 * 128].reshape(NCORES, BSH, NS, 7, 128)
        .transpose(0, 3, 2, 1, 4)
    )  # [NCORES, 7, NS, BSH, 128]
    noise_s7 = np.ascontiguousarray(
        n16[:, :, :, 7 * 128 :].transpose(0, 2, 1, 3))  # [NCORES, NS, BSH, 104]

    g3, mcat, bias_vec = _host_precompute(decay_param, conv_w, conv_b, out_w, out_b)

    # per-sample feedback scale: sigmoid(x . fb_w + fb_b) * sqrt(dt)/NS
    fb_w = np.asarray(fb_w, np.float32).reshape(IN)
    fb_b = float(np.asarray(fb_b, np.float32).reshape(-1)[0])
    z = x @ fb_w + fb_b
    cvec = (1.0 / (1.0 + np.exp(-z, dtype=np.float64))) * (np.sqrt(1.0 / NS) / NS)
    cvec = cvec.reshape(B).astype(np.float32)

    nc = _build_program()

    in_maps = []
    for c in range(NCORES):
        sl = slice(c * BSH, (c + 1) * BSH)
        cblk = np.zeros((128, 611), F16_NP)
        cblk[:, 0:3] = g3
        cblk[:, 3:99] = np.tile(cvec[sl], 3).reshape(1, 96)
        cblk[0, 99:611] = bias_vec
        in_maps.append(
            {
                "noise_sh": noise_q[c],
                "noise_s7": noise_s7[c],
                "cblk": np.ascontiguousarray(cblk),
                "mcat": mcat,
            }
        )

    res = run_bass_kernel_spmd(nc, in_maps, core_ids=list(range(NCORES)),
                               trace=_trace)
    LAST_RUN = res
    out = np.concatenate([m["out"] for m in res.results], axis=0)
    return out.astype(np.float32)


# revision 26
# speedup vs baseline: 1.0620x; 1.0553x over previous
"""Trainium2 Bass kernel for nn_BICEPNeuralLayer.

Math: the reference module (Euler-Maruyama SDE scan -> Conv1d over time ->
time-mean -> linear projection) is LINEAR in the noise tensor, so the whole
pipeline collapses algebraically:

  paths[t] = c_b * sum_s retain^(t-s) eps_s          (c_b = feedback_b*sqrt(dt))
  mean_t(conv(paths)) folds to per-timestep weights on eps:
     out[b] = (c_b/NS) * (Tsum @ A[b] - T0 @ L[b] - T2 @ F[b]) + bias
  A[b,i] = sum_s gA[s] noise[b,s,i],   gA[s] = (1-retain^(NS-s))/(1-retain)
  L[b,i] = sum_s retain^(NS-1-s) noise[b,s,i]
  F[b,i] = noise[b,0,i]
  Tsum = out_w @ (W0+W1+W2), T0 = out_w @ W0, T2 = out_w @ W2  (Wk = conv_w[:,:,k])
  bias  = out_w @ conv_b + out_b

Device work per core (pure data parallel over batch, 32 samples/core):
  The noise shard is pre-transposed on the host to chunk-major layout
  [q][s][b][i] (i padded 1000->1024, 8 chunks of 128 features) so every DMA
  descriptor is an 8 KB sequential DRAM run (~410 GB/s vs ~295 GB/s for the
  strided [b][s][i] order). mcat (the three folded [OUT, P] matrices) is cut
  into per-chunk slices interleaved with the noise chunks on the same queue,
  so stage 2 accumulates chunk-by-chunk behind the stream instead of
  serializing at the end. All small constants are host-pre-broadcast into a
  single [128, 611] fp32 block = one line-rate DMA (tiny broadcast transfers
  otherwise stall the ring for ~2 us at <40 GB/s).

  A 10-matmul warmup burst on zeroed scratch runs during the initial DMA
  window: the PE HAM clock-gate only reaches 2.4 GHz after ~3.4 us of
  sustained busy, and the real per-chunk bursts are too short to ever get
  there on their own (everything measured 2x slow at 1.2 GHz without this).

  per chunk q (software-pipelined: s1(0), s1(1), s2(0), s1(2), s2(1), ...):
    stage 1: 32 matmuls lhsT=noise[q][:,b,:] (fp16, FWL) rhs=g3[128,3]
             -> psum[i, (b,{A,L,F})]
    V build: DVE reorder (b,v)->(v,b) fused with the per-sample feedback
             scale c_b (host-precomputed sigmoid)
    stage 2: 3 accumulating matmuls lhsT=V[128i x 32b] rhs=mcat[128i x 512j]
             -> psum[32b, 512j]
  tail: add bias, store [32, 512] fp16 (host upcasts).
"""

import sys

if "/opt/trn_rl_repo" not in sys.path:
    sys.path.insert(0, "/opt/trn_rl_repo")

from contextlib import ExitStack

import numpy as np

import concourse.bass as bass
import concourse.tile as tile
from concourse import mybir
from concourse.bass_utils import run_bass_kernel_spmd

B, IN, OUT, P, NS = 256, 1024, 512, 1000, 128
NCORES = 8
BSH = B // NCORES  # 32 samples per core
NQ = 8             # feature chunks of 128 (P padded 1000 -> 1024)
PPAD = NQ * 128
NWARM = 10         # HAM warmup matmuls (N=512 each)

F32 = mybir.dt.float32
F16 = mybir.dt.float16
F16_NP = mybir.dt.np(F16)

_CACHE = {}

LAST_RUN = None  # BassKernelResults of the most recent execution (for test.py)


def _split_sync_waits(nc: bass.Bass, max_waits: int = 1) -> int:
    """Walrus in this container accepts at most one sync-wait command per
    instruction. Tile emits instructions (notably the epilogue Drain and any
    op depending on two DMA queues) with several waits. Split the surplus
    onto single-wait NoOps inserted just before, on the same engine, which
    is semantically identical for sem-ge waits."""
    nid = 0
    for fn in nc.m.functions:
        for bb in fn.blocks:
            insts = list(bb.instructions)
            out, changed = [], False
            for inst in insts:
                si = inst.sync_info
                if si is not None and si.on_wait and len(si.on_wait) > max_waits:
                    waits = list(si.on_wait)
                    extra, keep = waits[:-max_waits], waits[-max_waits:]
                    for w in extra:
                        nid += 1
                        out.append(
                            mybir.InstNoOp(
                                name=f"waitsplit-{nid}",
                                sync_info=mybir.SyncInfo(on_wait=[w], on_update=[]),
                                bass_nofuse=True,
                                engine=inst.engine,
                            )
                        )
                    inst.sync_info = mybir.SyncInfo(
                        on_wait=keep, on_update=list(si.on_update)
                    )
                    changed = True
                out.append(inst)
            if changed:
                bb.instructions = out
    return nid


def _build_program() -> bass.Bass:
    if "nc" in _CACHE:
        return _CACHE["nc"]

    nc = bass.Bass()

    noise_d = nc.dram_tensor("noise_sh", [NQ, NS, BSH, 128], F16,
                             kind="ExternalInput")
    # packed consts block: cols [0:3] g3 (as f32), [3:99] cvec broadcast,
    # [99:611] bias broadcast (meaningful on partitions 0:32)
    cblk_d = nc.dram_tensor("cblk", [128, 611], F32, kind="ExternalInput")
    mcat_d = nc.dram_tensor("mcat", [128, 3 * NQ, OUT], F16, kind="ExternalInput")
    out_d = nc.dram_tensor("out", [BSH, OUT], F16, kind="ExternalOutput")

    with ExitStack() as ctx:
        tc = ctx.enter_context(tile.TileContext(nc))
        consts = ctx.enter_context(tc.tile_pool(name="consts", bufs=1))
        npool = ctx.enter_context(tc.tile_pool(name="noise", bufs=NQ))
        vpool = ctx.enter_context(tc.tile_pool(name="v", bufs=1))
        ps1 = ctx.enter_context(tc.tile_pool(name="ps1", bufs=4, space="PSUM"))
        ps2 = ctx.enter_context(tc.tile_pool(name="ps2", bufs=1, space="PSUM"))
        wps = ctx.enter_context(tc.tile_pool(name="wps", bufs=1, space="PSUM"))

        # ---- HAM warmup: zeroed scratch matmuls, no data dependencies ----
        warm_sb = consts.tile([128, 512], F16, tag="warm")
        nc.vector.memset(warm_sb[:], 0.0)
        warm_ps = wps.tile([128, 512], F32, tag="warmps")
        for _ in range(NWARM):
            nc.tensor.matmul(warm_ps[:], lhsT=warm_sb[:, 0:128], rhs=warm_sb[:],
                             start=True, stop=True)

        # ---- consts: one line-rate transfer, first on the SP ring ----
        cblk_sb = consts.tile([128, 611], F32, tag="cblk")
        nc.sync.dma_start(out=cblk_sb[:], in_=cblk_d[:])
        c_sb = cblk_sb[:, 3:99]
        g3_sb = consts.tile([NS, 3], F16, tag="g3")
        nc.scalar.copy(g3_sb[:], cblk_sb[:, 0:3])
        # bias as fp16 row + a column of ones: the bias enters ps_out as an
        # early K=1 accumulating matmul (ones.T @ bias_row), so the final
        # psum->sbuf stores are pure casts split across DVE and ACT
        bias16 = consts.tile([1, OUT], F16, tag="bias16")
        nc.scalar.copy(bias16[:], cblk_sb[0:1, 99:611])
        ones_sb = consts.tile([1, BSH], F16, tag="ones")
        nc.vector.memset(ones_sb[:], 1.0)

        # ---- the big stream: noise chunk q (1 MB, fully sequential in DRAM)
        # interleaved with mcat slice q (393 KB) on the SP ring. Stage-2 for
        # chunk q only needs bytes that arrived with chunk q, so compute
        # chases the stream and almost nothing is left after the last byte.
        mcat_sb = consts.tile([128, 3 * NQ, OUT], F16, tag="mcat")
        noise_t = []
        HB = BSH // 2
        for q in range(NQ):
            t = npool.tile([NS, BSH, 128], F16, name=f"noise{q}", tag="noise")
            # mcat slice q AFTER noise chunk q: the last chunk's V-build path
            # then starts as early as possible and the final mcat slice's
            # completion receipt hides under it.
            nc.sync.dma_start(out=t[:], in_=noise_d[q])
            noise_t.append(t)
            nc.sync.dma_start(out=mcat_sb[:, q * 3 : (q + 1) * 3, :],
                              in_=mcat_d[:][:, q * 3 : (q + 1) * 3, :])

        # ---- per-chunk pipeline, software-pipelined by one chunk: the PE
        # stream is s1(0), s1(1), s2(0), s1(2), s2(1), ... so the DVE V-build
        # of chunk q overlaps stage-1 of chunk q+1 instead of stalling the PE.
        ps_out = ps2.tile([BSH, OUT], F32, tag="ps2")
        v_t = [vpool.tile([128, 3 * BSH], F16, name=f"v{q}", tag=f"v{q}")
               for q in range(NQ)]

        def stage1(q):
            pt = ps1.tile([128, 3 * BSH], F32, name=f"ps1_{q}", tag="ps1")
            for b in range(BSH):
                nc.tensor.matmul(
                    pt[:, b * 3 : b * 3 + 3],
                    lhsT=noise_t[q][:, b, :],
                    rhs=g3_sb[:],
                    start=True,
                    stop=True,
                )
            # psum -> V (fp16): reorder (b,v) -> (v,b) and fold the
            # per-sample feedback scale c_b in (c columns follow V layout)
            src = pt[:].rearrange("p (b v) -> p v b", v=3)
            dst = v_t[q][:].rearrange("p (v b) -> p v b", v=3)
            csrc = c_sb.rearrange("p (v b) -> p v b", v=3)
            nc.vector.tensor_mul(dst, src, csrc)

        def stage2(q):
            for v in range(3):
                t = q * 3 + v
                nc.tensor.matmul(
                    ps_out[:],
                    lhsT=v_t[q][:, v * BSH : (v + 1) * BSH],
                    rhs=mcat_sb[:, t, :],
                    start=False,
                    stop=(t == 3 * NQ - 1),
                )

        # bias into ps_out (opens the accumulation group)
        nc.tensor.matmul(ps_out[:], lhsT=ones_sb[:], rhs=bias16[:],
                         start=True, stop=False)

        stage1(0)
        for q in range(1, NQ):
            stage1(q)
            stage2(q - 1)
        stage2(NQ - 1)

        # ---- store fp16 (bias already accumulated in psum): pure casts,
        # split DVE/ACT so the two halves overlap ----
        out_sb = consts.tile([BSH, OUT], F16, tag="outsb")
        nc.vector.tensor_scalar_add(out_sb[:, 0:256], ps_out[:, 0:256], 0.0)
        nc.scalar.copy(out_sb[:, 256:512], ps_out[:, 256:512])
        nc.sync.dma_start(out=out_d[:], in_=out_sb[:])

    _split_sync_waits(nc)
    _CACHE["nc"] = nc
    return nc


def _host_precompute(decay_param, conv_w, conv_b, out_w, out_b):
    dp = float(np.asarray(decay_param).reshape(-1)[0])
    decay = 0.5 / (1.0 + np.exp(-dp))
    dt = 1.0 / NS
    retain = 1.0 - decay * dt

    s = np.arange(NS, dtype=np.float64)
    gA = (1.0 - retain ** (NS - s)) / (1.0 - retain)
    gL = retain ** (NS - 1 - s)
    g3 = np.zeros((NS, 3), np.float32)
    g3[:, 0] = gA
    g3[:, 1] = gL
    g3[0, 2] = 1.0

    conv_w = np.asarray(conv_w, np.float32)
    out_w = np.asarray(out_w, np.float32)
    w_sum = conv_w.sum(axis=2)
    t_sum = out_w @ w_sum              # [OUT, P]
    t0 = out_w @ conv_w[:, :, 0]
    t2 = out_w @ conv_w[:, :, 2]
    r = np.stack([t_sum, -t0, -t2])    # [3, OUT, P]
    r_pad = np.zeros((3, OUT, PPAD), np.float32)
    r_pad[:, :, :P] = r
    # mcat[p, q*3+v, j] = r[v, j, q*128+p]  (q-major: per-chunk slices)
    mcat = r_pad.reshape(3, OUT, NQ, 128).transpose(3, 2, 0, 1)  # [128, NQ, 3, OUT]
    mcat = np.ascontiguousarray(mcat.reshape(128, 3 * NQ, OUT).astype(F16_NP))

    bias_vec = (
        out_w @ np.asarray(conv_b, np.float32)
        + np.asarray(out_b, np.float32).reshape(OUT)
    )
    return g3, mcat, bias_vec


def kernel(x, noise, fb_w, fb_b, decay_param, conv_w, conv_b, out_w, out_b,
           _trace=False):
    global LAST_RUN

    x = np.asarray(x, np.float32)
    # chunk-major, feature-padded, per-core noise layout [core][q][s][b][i]:
    # every DMA descriptor reads an 8 KB sequential DRAM run.
    n16 = np.zeros((B, NS, PPAD), F16_NP)
    n16[:, :, :P] = np.asarray(noise, np.float32).astype(F16_NP)
    noise_q = np.ascontiguousarray(
        n16.reshape(NCORES, BSH, NS, NQ, 128).transpose(0, 3, 2, 1, 4)
    )  # [NCORES, NQ, NS, BSH, 128]

    g3, mcat, bias_vec = _host_precompute(decay_param, conv_w, conv_b, out_w, out_b)

    # per-sample feedback scale: sigmoid(x . fb_w + fb_b) * sqrt(dt)/NS
    fb_w = np.asarray(fb_w, np.float32).reshape(IN)
    fb_b = float(np.asarray(fb_b, np.float32).reshape(-1)[0])
    z = x @ fb_w + fb_b
    cvec = (1.0 / (1.0 + np.exp(-z, dtype=np.float64))) * (np.sqrt(1.0 / NS) / NS)
    cvec = cvec.reshape(B).astype(np.float32)

    nc = _build_program()

    in_maps = []
    for c in range(NCORES):
        sl = slice(c * BSH, (c + 1) * BSH)
        cblk = np.zeros((128, 611), np.float32)
        cblk[:, 0:3] = g3
        cblk[:, 3:99] = np.tile(cvec[sl], 3).reshape(1, 96)
        cblk[0:BSH, 99:611] = bias_vec.reshape(1, OUT)
        in_maps.append(
            {
                "noise_sh": noise_q[c],
                "cblk": np.ascontiguousarray(cblk),
                "mcat": mcat,
            }
        )

    res = run_bass_kernel_spmd(nc, in_maps, core_ids=list(range(NCORES)),
                               trace=_trace)
    LAST_RUN = res
    out = np.concatenate([m["out"] for m in res.results], axis=0)
    return out.astype(np.float32)


# revision 28
# speedup vs baseline: 1.1137x; 1.0487x over previous
"""Trainium2 Bass kernel for nn_BICEPNeuralLayer.

Math: the reference module (Euler-Maruyama SDE scan -> Conv1d over time ->
time-mean -> linear projection) is LINEAR in the noise tensor, so the whole
pipeline collapses algebraically:

  paths[t] = c_b * sum_s retain^(t-s) eps_s          (c_b = feedback_b*sqrt(dt))
  mean_t(conv(paths)) folds to per-timestep weights on eps:
     out[b] = (c_b/NS) * (Tsum @ A[b] - T0 @ L[b] - T2 @ F[b]) + bias
  A[b,i] = sum_s gA[s] noise[b,s,i],   gA[s] = (1-retain^(NS-s))/(1-retain)
  L[b,i] = sum_s retain^(NS-1-s) noise[b,s,i]
  F[b,i] = noise[b,0,i]
  Tsum = out_w @ (W0+W1+W2), T0 = out_w @ W0, T2 = out_w @ W2  (Wk = conv_w[:,:,k])
  bias  = out_w @ conv_b + out_b

Device work per core (pure data parallel over batch, 32 samples/core):
  The noise shard is pre-transposed on the host to chunk-major layout
  [q][s][b][i] (i padded 1000->1024, 8 chunks of 128 features) so every DMA
  descriptor is an 8 KB sequential DRAM run (~410 GB/s vs ~295 GB/s for the
  strided [b][s][i] order). mcat (the three folded [OUT, P] matrices) is cut
  into per-chunk slices interleaved with the noise chunks on the same queue,
  so stage 2 accumulates chunk-by-chunk behind the stream instead of
  serializing at the end. All small constants are host-pre-broadcast into a
  single [128, 611] fp32 block = one line-rate DMA (tiny broadcast transfers
  otherwise stall the ring for ~2 us at <40 GB/s).

  A 10-matmul warmup burst on zeroed scratch runs during the initial DMA
  window: the PE HAM clock-gate only reaches 2.4 GHz after ~3.4 us of
  sustained busy, and the real per-chunk bursts are too short to ever get
  there on their own (everything measured 2x slow at 1.2 GHz without this).

  per chunk q (software-pipelined: s1(0), s1(1), s2(0), s1(2), s2(1), ...):
    stage 1: 32 matmuls lhsT=noise[q][:,b,:] (fp16, FWL) rhs=g3[128,3]
             -> psum[i, (b,{A,L,F})]
    V build: DVE reorder (b,v)->(v,b) fused with the per-sample feedback
             scale c_b (host-precomputed sigmoid)
    stage 2: 3 accumulating matmuls lhsT=V[128i x 32b] rhs=mcat[128i x 512j]
             -> psum[32b, 512j]
  tail: add bias, store [32, 512] fp16 (host upcasts).
"""

import sys

if "/opt/trn_rl_repo" not in sys.path:
    sys.path.insert(0, "/opt/trn_rl_repo")

from contextlib import ExitStack

import numpy as np

import concourse.bass as bass
import concourse.tile as tile
from concourse import mybir
from concourse.bass_utils import run_bass_kernel_spmd

B, IN, OUT, P, NS = 256, 1024, 512, 1000, 128
NCORES = 8
BSH = B // NCORES  # 32 samples per core
NQ = 8             # feature chunks of 128 (P padded 1000 -> 1024)
PPAD = NQ * 128
NWARM = 10         # HAM warmup matmuls (N=512 each)

F32 = mybir.dt.float32
F16 = mybir.dt.float16
F16_NP = mybir.dt.np(F16)

_CACHE = {}

LAST_RUN = None  # BassKernelResults of the most recent execution (for test.py)


def _split_sync_waits(nc: bass.Bass, max_waits: int = 1) -> int:
    """Walrus in this container accepts at most one sync-wait command per
    instruction. Tile emits instructions (notably the epilogue Drain and any
    op depending on two DMA queues) with several waits. Split the surplus
    onto single-wait NoOps inserted just before, on the same engine, which
    is semantically identical for sem-ge waits."""
    nid = 0
    for fn in nc.m.functions:
        for bb in fn.blocks:
            insts = list(bb.instructions)
            out, changed = [], False
            for inst in insts:
                si = inst.sync_info
                if si is not None and si.on_wait and len(si.on_wait) > max_waits:
                    waits = list(si.on_wait)
                    extra, keep = waits[:-max_waits], waits[-max_waits:]
                    for w in extra:
                        nid += 1
                        out.append(
                            mybir.InstNoOp(
                                name=f"waitsplit-{nid}",
                                sync_info=mybir.SyncInfo(on_wait=[w], on_update=[]),
                                bass_nofuse=True,
                                engine=inst.engine,
                            )
                        )
                    inst.sync_info = mybir.SyncInfo(
                        on_wait=keep, on_update=list(si.on_update)
                    )
                    changed = True
                out.append(inst)
            if changed:
                bb.instructions = out
    return nid


def _build_program() -> bass.Bass:
    if "nc" in _CACHE:
        return _CACHE["nc"]

    nc = bass.Bass()

    noise_d = nc.dram_tensor("noise_sh", [NQ, NS, BSH, 128], F16,
                             kind="ExternalInput")
    # packed consts block: cols [0:3] g3 (as f32), [3:99] cvec broadcast,
    # [99:611] bias broadcast (meaningful on partitions 0:32)
    cblk_d = nc.dram_tensor("cblk", [128, 611], F32, kind="ExternalInput")
    mcat_d = nc.dram_tensor("mcat", [128, 3 * NQ, OUT], F16, kind="ExternalInput")
    out_d = nc.dram_tensor("out", [BSH, OUT], F16, kind="ExternalOutput")

    with ExitStack() as ctx:
        tc = ctx.enter_context(tile.TileContext(nc))
        consts = ctx.enter_context(tc.tile_pool(name="consts", bufs=1))
        npool = ctx.enter_context(tc.tile_pool(name="noise", bufs=NQ))
        vpool = ctx.enter_context(tc.tile_pool(name="v", bufs=1))
        ps1 = ctx.enter_context(tc.tile_pool(name="ps1", bufs=4, space="PSUM"))
        ps2 = ctx.enter_context(tc.tile_pool(name="ps2", bufs=1, space="PSUM"))
        wps = ctx.enter_context(tc.tile_pool(name="wps", bufs=1, space="PSUM"))

        # ---- HAM warmup: zeroed scratch matmuls, no data dependencies ----
        warm_sb = consts.tile([128, 512], F16, tag="warm")
        nc.vector.memset(warm_sb[:], 0.0)
        warm_ps = wps.tile([128, 512], F32, tag="warmps")
        for _ in range(NWARM):
            nc.tensor.matmul(warm_ps[:], lhsT=warm_sb[:, 0:128], rhs=warm_sb[:],
                             start=True, stop=True)

        # ---- consts: one line-rate transfer, first on the SP ring ----
        cblk_sb = consts.tile([128, 611], F32, tag="cblk")
        nc.sync.dma_start(out=cblk_sb[:], in_=cblk_d[:])
        c_sb = cblk_sb[:, 3:99]
        g3_sb = consts.tile([NS, 3], F16, tag="g3")
        nc.scalar.copy(g3_sb[:], cblk_sb[:, 0:3])
        # bias as fp16 row + a column of ones: the bias enters ps_out as an
        # early K=1 accumulating matmul (ones.T @ bias_row), so the final
        # psum->sbuf stores are pure casts split across DVE and ACT
        bias16 = consts.tile([1, OUT], F16, tag="bias16")
        nc.scalar.copy(bias16[:], cblk_sb[0:1, 99:611])
        ones_sb = consts.tile([1, BSH], F16, tag="ones")
        nc.vector.memset(ones_sb[:], 1.0)

        # ---- the big stream: noise chunk q (1 MB, fully sequential in DRAM)
        # interleaved with mcat slice q (393 KB) on the SP ring. Stage-2 for
        # chunk q only needs bytes that arrived with chunk q, so compute
        # chases the stream and almost nothing is left after the last byte.
        mcat_sb = consts.tile([128, 3 * NQ, OUT], F16, tag="mcat")
        noise_t = []
        HB = BSH // 2
        for q in range(NQ):
            t = npool.tile([NS, BSH, 128], F16, name=f"noise{q}", tag="noise")
            # mcat slice q AFTER noise chunk q: the last chunk's V-build path
            # then starts as early as possible and the final mcat slice's
            # completion receipt hides under it.
            nc.sync.dma_start(out=t[:], in_=noise_d[q])
            noise_t.append(t)
            nc.sync.dma_start(out=mcat_sb[:, q * 3 : (q + 1) * 3, :],
                              in_=mcat_d[:][:, q * 3 : (q + 1) * 3, :])

        # ---- per-chunk pipeline, software-pipelined by one chunk: the PE
        # stream is s1(0), s1(1), s2(0), s1(2), s2(1), ... so the DVE V-build
        # of chunk q overlaps stage-1 of chunk q+1 instead of stalling the PE.
        ps_out = ps2.tile([BSH, OUT], F32, tag="ps2")
        v_t = [vpool.tile([128, 3 * BSH], F16, name=f"v{q}", tag=f"v{q}")
               for q in range(NQ)]

        def stage1(q):
            pt = ps1.tile([128, 3 * BSH], F32, name=f"ps1_{q}", tag="ps1")
            for b in range(BSH):
                nc.tensor.matmul(
                    pt[:, b * 3 : b * 3 + 3],
                    lhsT=noise_t[q][:, b, :],
                    rhs=g3_sb[:],
                    start=True,
                    stop=True,
                )
            # psum -> V (fp16): reorder (b,v) -> (v,b) and fold the
            # per-sample feedback scale c_b in (c columns follow V layout)
            src = pt[:].rearrange("p (b v) -> p v b", v=3)
            dst = v_t[q][:].rearrange("p (v b) -> p v b", v=3)
            csrc = c_sb.rearrange("p (v b) -> p v b", v=3)
            nc.vector.tensor_mul(dst, src, csrc)

        def stage2(q):
            for v in range(3):
                t = q * 3 + v
                nc.tensor.matmul(
                    ps_out[:],
                    lhsT=v_t[q][:, v * BSH : (v + 1) * BSH],
                    rhs=mcat_sb[:, t, :],
                    start=False,
                    stop=(t == 3 * NQ - 1),
                )

        # bias into ps_out (opens the accumulation group)
        nc.tensor.matmul(ps_out[:], lhsT=ones_sb[:], rhs=bias16[:],
                         start=True, stop=False)

        stage1(0)
        for q in range(1, NQ):
            stage1(q)
            stage2(q - 1)
        stage2(NQ - 1)

        # ---- store fp16 (bias already accumulated in psum): pure casts,
        # split DVE/ACT so the two halves overlap ----
        out_sb = consts.tile([BSH, OUT], F16, tag="outsb")
        nc.vector.tensor_scalar_add(out_sb[:, 0:256], ps_out[:, 0:256], 0.0)
        nc.scalar.copy(out_sb[:, 256:512], ps_out[:, 256:512])
        nc.sync.dma_start(out=out_d[:], in_=out_sb[:])

    _split_sync_waits(nc)
    _CACHE["nc"] = nc
    return nc


def _host_precompute(decay_param, conv_w, conv_b, out_w, out_b):
    dp = float(np.asarray(decay_param).reshape(-1)[0])
    decay = 0.5 / (1.0 + np.exp(-dp))
    dt = 1.0 / NS
    retain = 1.0 - decay * dt

    s = np.arange(NS, dtype=np.float64)
    gA = (1.0 - retain ** (NS - s)) / (1.0 - retain)
    gL = retain ** (NS - 1 - s)
    g3 = np.zeros((NS, 3), np.float32)
    g3[:, 0] = gA
    g3[:, 1] = gL
    g3[0, 2] = 1.0

    conv_w = np.asarray(conv_w, np.float32)
    out_w = np.asarray(out_w, np.float32)
    w_sum = conv_w.sum(axis=2)
    t_sum = out_w @ w_sum              # [OUT, P]
    t0 = out_w @ conv_w[:, :, 0]
    t2 = out_w @ conv_w[:, :, 2]
    r = np.stack([t_sum, -t0, -t2])    # [3, OUT, P]
    r_pad = np.zeros((3, OUT, PPAD), np.float32)
    r_pad[:, :, :P] = r
    # mcat[p, q*3+v, j] = r[v, j, q*128+p]  (q-major: per-chunk slices)
    mcat = r_pad.reshape(3, OUT, NQ, 128).transpose(3, 2, 0, 1)  # [128, NQ, 3, OUT]
    mcat = np.ascontiguousarray(mcat.reshape(128, 3 * NQ, OUT).astype(F16_NP))

    bias_vec = (
        out_w @ np.asarray(conv_b, np.float32)
        + np.asarray(out_b, np.float32).reshape(OUT)
    )
    return g3, mcat, bias_vec


def kernel(x, noise, fb_w, fb_b, decay_param, conv_w, conv_b, out_w, out_b,
           _trace=False):
    global LAST_RUN

    x = np.asarray(x, np.float32)
    # chunk-major, feature-padded, per-core noise layout [core][q][s][b][i]:
    # every DMA descriptor reads an 8 KB sequential DRAM run.
    n16 = np.zeros((B, NS, PPAD), F16_NP)
    n16[:, :, :P] = np.asarray(noise, np.float32).astype(F16_NP)
    noise_q = np.ascontiguousarray(
        n16.reshape(NCORES, BSH, NS, NQ, 128).transpose(0, 3, 2, 1, 4)
    )  # [NCORES, NQ, NS, BSH, 128]

    g3, mcat, bias_vec = _host_precompute(decay_param, conv_w, conv_b, out_w, out_b)

    # per-sample feedback scale: sigmoid(x . fb_w + fb_b) * sqrt(dt)/NS
    fb_w = np.asarray(fb_w, np.float32).reshape(IN)
    fb_b = float(np.asarray(fb_b, np.float32).reshape(-1)[0])
    z = x @ fb_w + fb_b
    cvec = (1.0 / (1.0 + np.exp(-z, dtype=np.float64))) * (np.sqrt(1.0 / NS) / NS)
    cvec = cvec.reshape(B).astype(np.float32)

    nc = _build_program()

    in_maps = []
    for c in range(NCORES):
        sl = slice(c * BSH, (c + 1) * BSH)
        cblk = np.zeros((128, 611), np.float32)
        cblk[:, 0:3] = g3
        cblk[:, 3:99] = np.tile(cvec[sl], 3).reshape(1, 96)
        cblk[0:BSH, 99:611] = bias_vec.reshape(1, OUT)
        in_maps.append(
            {
                "noise_sh": noise_q[c],
                "cblk": np.ascontiguousarray(cblk),
                "mcat": mcat,
            }
        )

    res = run_bass_kernel_spmd(nc, in_maps, core_ids=list(range(NCORES)),
                               trace=_trace)
    LAST_RUN = res
    out = np.concatenate([m["out"] for m in res.results], axis=0)
    return out.astype(np.float32)
